# revision 2
# baseline (speedup 1.0000x reference)
"""Multi-head attention (B=2, S=1024, D=768, H=12) on 8 TRN2 NeuronCores. v2.

Sharding: batch x head-group. Core c handles batch b = c // 4 and heads
3*(c%4) .. 3*(c%4)+2. Host sums the 4 partials per batch and adds bo.

v2 changes vs v1:
- f16 everywhere on device (qkv, weights, mask, intermediates, out):
  halves HBM traffic and enables 2x DVE mode for the mask op.
- The additive attention mask is shipped as exp(mask) (host precompute)
  and applied as a f16 multiply AFTER the exp (2x DVE) instead of an
  fp32 add on the PSUM logits before it (1x DVE).
- No ones-row in the q/k contraction: biases are added on the PSUM->SBUF
  copies via activation bias (per-partition). V keeps a 1-row ones tile
  (bias + softmax-denominator column), which costs a ~60-cycle matmul.
- Head 2's q/k projections are written twice (duplicated weight columns,
  free on the PE), so every S matmul (contraction 64) can be issued in
  pairs on distinct PE row-groups (0-63 / 64-127) and overlap 2x.
"""

import numpy as np

B, SQ, SK, D, H = 2, 1024, 1024, 768, 12
DH = D // H            # 64
HPC = 3                # heads per core
N_CORES = 8
GPB = 4                # head-groups (cores) per batch
KT = 6                 # k-tiles over contraction dim 768
NEG = -1.0e30

_CACHE = {}


def _build(repeats=1):
    import concourse.tile as tile
    import concourse.mybir as mybir
    from concourse import bacc

    f32 = mybir.dt.float32
    f16 = mybir.dt.float16
    AF = mybir.ActivationFunctionType

    nc = bacc.Bacc("TRN2", target_bir_lowering=False, debug=False,
                   num_devices=N_CORES)

    qT = nc.dram_tensor("qT", [D, SQ], f16, kind="ExternalInput").ap()
    kT = nc.dram_tensor("kT", [D, SK], f16, kind="ExternalInput").ap()
    vT = nc.dram_tensor("vT", [D + 1, SK], f16, kind="ExternalInput").ap()
    # WqkA cols: 0:128 Wq(h01), 128:256 Wq(h2,h2), 256:384 Wk(h01),
    # 384:512 Wk(h2,h2); q cols pre-scaled by 1/sqrt(DH)
    WqkA = nc.dram_tensor("WqkA", [D, 512], f16, kind="ExternalInput").ap()
    # WvA: per j: cols j*65:j*65+64 = Wv(head j), col j*65+64 = ones col;
    # row 768 = bias row (bv | 1)
    WvA = nc.dram_tensor("WvA", [D + 1, 196], f16, kind="ExternalInput").ap()
    wo01_d = nc.dram_tensor("wo01", [128, D], f16, kind="ExternalInput").ap()
    wo2_d = nc.dram_tensor("wo2", [DH, D], f16, kind="ExternalInput").ap()
    # biasA cols: 0 = bq(h01)*scale, 1 = bq(h2,h2)*scale, 2 = bk(h01),
    # 3 = bk(h2,h2)
    biasA = nc.dram_tensor("biasA", [128, 4], f32, kind="ExternalInput").ap()
    padc = nc.dram_tensor("padc", [128, 8], f32, kind="ExternalInput").ap()
    ones64 = nc.dram_tensor("ones64", [1, DH], f16, kind="ExternalInput").ap()
    # exp(mask)^T per head: [HPC, SK, SQ]
    emT = nc.dram_tensor("emT", [HPC, SK, SQ], f16, kind="ExternalInput").ap()
    out_d = nc.dram_tensor("out", [SQ, D], f16, kind="ExternalOutput").ap()

    with tile.TileContext(nc) as tc:
        with (
            tc.tile_pool(name="consts", bufs=1) as cp,
            tc.tile_pool(name="xt", bufs=10) as xtp,
            tc.tile_pool(name="qk", bufs=1) as qkp,
            tc.tile_pool(name="vv", bufs=1) as vvp,
            tc.tile_pool(name="mask", bufs=6) as mkp,
            tc.tile_pool(name="pr", bufs=4) as prp,
            tc.tile_pool(name="pt", bufs=10) as ptp,
            tc.tile_pool(name="norm", bufs=1) as nmp,
            tc.tile_pool(name="tmp", bufs=2) as tmp,
            tc.tile_pool(name="outs", bufs=3) as otp,
            tc.tile_pool(name="ps", bufs=2, space="PSUM") as ps,
            tc.tile_pool(name="cx", bufs=2, space="PSUM") as cxp,
        ):
            # ---- constants (loaded once) ----
            wqk = []
            for t in range(KT):
                w1 = cp.tile([128, 512], f16, tag=f"wqk{t}")
                nc.sync.dma_start(w1[:], WqkA[t * 128:(t + 1) * 128, :])
                wqk.append(w1)
            wv = []
            for t in range(KT):
                w3 = cp.tile([128, 196], f16, tag=f"wv{t}")
                nc.sync.dma_start(w3[:], WvA[t * 128:(t + 1) * 128, :])
                wv.append(w3)
            wvb = cp.tile([1, 196], f16, tag="wvb")
            nc.sync.dma_start(wvb[:], WvA[D:D + 1, :])
            biasT = cp.tile([128, 4], f32, tag="biasT")
            nc.sync.dma_start(biasT[:], biasA)
            o64 = cp.tile([1, DH], f16, tag="o64")
            nc.sync.dma_start(o64[:], ones64)
            wo01 = cp.tile([128, D], f16, tag="wo01")
            nc.sync.dma_start(wo01[:], wo01_d)
            wo2 = cp.tile([DH, D], f16, tag="wo2")
            nc.sync.dma_start(wo2[:], wo2_d)

            def load_x(x_dram, with_ones):
                ts_ = []
                for t in range(KT):
                    xt_t = xtp.tile([128, SQ], f16, tag="xt")
                    nc.sync.dma_start(xt_t[:], x_dram[t * 128:(t + 1) * 128, :])
                    ts_.append(xt_t)
                if with_ones:
                    xo = xtp.tile([1, SQ], f16, tag="xones")
                    nc.sync.dma_start(xo[:], x_dram[D:D + 1, :])
                    ts_.append(xo)
                return ts_

            for _rep in range(repeats):
                pad = cp.tile([128, 8], f32, tag="pad")
                nc.sync.dma_start(pad[:], padc)

                # ---- q/k projections -> [128, SQ] f16, channel-partition ----
                def proj_one(xts, col0, bcol, tag):
                    dst = qkp.tile([128, SQ], f16, tag=tag)
                    pps = ps.tile([128, SQ], f32, tag="ps")
                    for t in range(KT):
                        lhs = wqk[t][:, col0: col0 + 128]
                        for n in range(2):
                            nc.tensor.matmul(
                                pps[:, n * 512:(n + 1) * 512],
                                lhs, xts[t][:, n * 512:(n + 1) * 512],
                                start=(t == 0), stop=(t == KT - 1))
                    nc.scalar.add(dst[:], pps[:], biasT[:, bcol:bcol + 1])
                    return dst

                qx = load_x(qT, False)
                q_c0 = proj_one(qx, 0, 0, "q0")
                q_c1 = proj_one(qx, 128, 1, "q1")
                kx = load_x(kT, False)
                k_c0 = proj_one(kx, 256, 2, "k0")
                k_c1 = proj_one(kx, 384, 3, "k1")

                # ---- v projection -> 8 x [128 tok, 196 ch] f16 ----
                vx = load_x(vT, True)
                vtiles = []
                for i in range(8):
                    vps = ps.tile([128, 196], f32, tag="ps")
                    for t in range(KT):
                        nc.tensor.matmul(
                            vps[:], vx[t][:, i * 128:(i + 1) * 128], wv[t][:],
                            start=(t == 0), stop=False)
                    nc.tensor.matmul(
                        vps[:], vx[KT][:, i * 128:(i + 1) * 128], wvb[:],
                        start=False, stop=True)
                    vt = vvp.tile([128, 196], f16, tag=f"v{i}")
                    nc.scalar.copy(vt[:], vps[:])
                    vtiles.append(vt)

                # ---- attention ----
                # rows(j): which 64-row group head j's q/k slice lives on,
                # per i-tile. h0: rows 0:64 of q_c0; h1: rows 64:128 of
                # q_c0; h2: alternates between halves of q_c1 (duplicated).
                def qk_slice(j, i):
                    if j == 0:
                        return q_c0[0:DH, :], k_c0[0:DH, :]
                    if j == 1:
                        return q_c0[DH:128, :], k_c0[DH:128, :]
                    if i % 2 == 0:
                        return q_c1[0:DH, :], k_c1[0:DH, :]
                    return q_c1[DH:128, :], k_c1[DH:128, :]

                pts = {}

                def emit_tile(j, i):
                    qs, ks = qk_slice(j, i)
                    sps = ps.tile([128, SQ], f32, tag="ps")
                    for n in range(2):
                        nc.tensor.matmul(
                            sps[:, n * 512:(n + 1) * 512],
                            ks[:, i * 128:(i + 1) * 128],
                            qs[:, n * 512:(n + 1) * 512],
                            start=True, stop=True)
                    ptr = prp.tile([128, SQ], f16, tag="ptr")
                    nc.scalar.activation(ptr[:], sps[:], AF.Exp,
                                         bias=pad[:, i:i + 1], scale=1.0)
                    mk = mkp.tile([128, SQ], f16, tag="mask")
                    nc.sync.dma_start(mk[:], emT[j, i * 128:(i + 1) * 128, :])
                    pt = ptp.tile([128, SQ], f16, tag="pt")
                    nc.vector.tensor_mul(pt[:], ptr[:], mk[:])
                    pts[(j, i)] = pt

                def emit_av(ctx_, j_, i_):
                    pt_ = pts[(j_, i_)]
                    for n in range(2):
                        nc.tensor.matmul(
                            ctx_[:, n * 512:(n + 1) * 512],
                            vtiles[i_][:, j_ * 65:j_ * 65 + 65],
                            pt_[:, n * 512:(n + 1) * 512],
                            start=(i_ == 0), stop=(i_ == 7))
                    del pts[(j_, i_)]

                cn01 = nmp.tile([128, SQ], f16, tag="cn01")
                cn2 = nmp.tile([DH, SQ], f16, tag="cn2")

                def emit_norm(j, ctx):
                    srow = tmp.tile([1, SQ], f16, tag="srow")
                    nc.scalar.copy(srow[:], ctx[DH:DH + 1, :])
                    rb = ps.tile([DH, SQ], f32, tag="ps")
                    for n in range(2):
                        nc.tensor.matmul(rb[:, n * 512:(n + 1) * 512],
                                         o64[:], srow[:, n * 512:(n + 1) * 512],
                                         start=True, stop=True)
                    rbr = tmp.tile([DH, SQ], f32, tag="rbr")
                    nc.vector.reciprocal(rbr[:], rb[:])
                    if j < 2:
                        nc.vector.tensor_mul(cn01[j * DH:(j + 1) * DH, :],
                                             ctx[0:DH, :], rbr[:])
                    else:
                        nc.vector.tensor_mul(cn2[:], ctx[0:DH, :], rbr[:])

                ctx0 = cxp.tile([65, SQ], f32, tag="cx")
                ctx1 = cxp.tile([65, SQ], f32, tag="cx")
                # h0/h1 interleaved: S pairs land on distinct PE row groups
                for i in range(8):
                    emit_tile(0, i)
                    emit_tile(1, i)
                    if i > 0:
                        emit_av(ctx0, 0, i - 1)
                        emit_av(ctx1, 1, i - 1)
                emit_av(ctx0, 0, 7)
                emit_av(ctx1, 1, 7)
                emit_norm(0, ctx0)
                emit_norm(1, ctx1)
                # h2: alternating row groups via duplicated q_c1/k_c1
                ctx2 = cxp.tile([65, SQ], f32, tag="cx")
                for i in range(8):
                    emit_tile(2, i)
                    if i > 0:
                        emit_av(ctx2, 2, i - 1)
                emit_av(ctx2, 2, 7)
                emit_norm(2, ctx2)

                # ---- output projection ----
                for t in range(8):
                    ops = ps.tile([128, D], f32, tag="ps")
                    for n0, nw in ((0, 512), (512, 256)):
                        nc.tensor.matmul(
                            ops[:, n0:n0 + nw],
                            cn01[:, t * 128:(t + 1) * 128],
                            wo01[:, n0:n0 + nw],
                            start=True, stop=False)
                        nc.tensor.matmul(
                            ops[:, n0:n0 + nw],
                            cn2[:, t * 128:(t + 1) * 128],
                            wo2[:, n0:n0 + nw],
                            start=False, stop=True)
                    ot = otp.tile([128, D], f16, tag=f"ot{t % 3}")
                    if t % 2 == 0:
                        nc.vector.tensor_copy(ot[:], ops[:])
                    else:
                        nc.scalar.copy(ot[:], ops[:])
                    nc.sync.dma_start(out_d[t * 128:(t + 1) * 128, :], ot[:])

    nc.compile()
    return nc


def prep_inputs(value, key, query, key_padding_mask, attn_mask,
                Wq, Wk, Wv, Wo, bq, bk, bv, bo):
    f = np.float32
    h = np.float16
    value = np.asarray(value, f)
    key = np.asarray(key, f)
    query = np.asarray(query, f)
    key_padding_mask = np.asarray(key_padding_mask)
    attn_mask = np.asarray(attn_mask, f)
    Wq, Wk, Wv, Wo = (np.asarray(w, f) for w in (Wq, Wk, Wv, Wo))
    bq, bk, bv = (np.asarray(x, f) for x in (bq, bk, bv))

    scale = f(1.0 / np.sqrt(DH))
    ones_row = np.ones((1, SK), h)
    xT = {}
    for b in range(B):
        xT[("q", b)] = np.ascontiguousarray(query[b].T).astype(h)
        xT[("k", b)] = np.ascontiguousarray(key[b].T).astype(h)
        xT[("v", b)] = np.concatenate(
            [np.ascontiguousarray(value[b].T).astype(h), ones_row])
    emT_all = np.exp(attn_mask.transpose(0, 1, 3, 2)).astype(h)
    pad_all = np.where(key_padding_mask, f(0), f(NEG)).astype(f)  # [B, SK]

    in_maps = []
    for c in range(N_CORES):
        b, g = divmod(c, GPB)
        h0 = g * HPC
        c01 = slice(h0 * DH, (h0 + 2) * DH)
        c2 = slice((h0 + 2) * DH, (h0 + 3) * DH)
        WqkA = np.zeros((D, 512), h)
        WqkA[:, 0:128] = (Wq[:, c01] * scale).astype(h)
        WqkA[:, 128:192] = (Wq[:, c2] * scale).astype(h)
        WqkA[:, 192:256] = WqkA[:, 128:192]
        WqkA[:, 256:384] = Wk[:, c01].astype(h)
        WqkA[:, 384:448] = Wk[:, c2].astype(h)
        WqkA[:, 448:512] = WqkA[:, 384:448]
        WvA = np.zeros((D + 1, 196), h)
        for j in range(HPC):
            hc = slice((h0 + j) * DH, (h0 + j + 1) * DH)
            WvA[:D, j * 65:j * 65 + DH] = Wv[:, hc].astype(h)
            WvA[D, j * 65:j * 65 + DH] = bv[hc].astype(h)
            WvA[D, j * 65 + DH] = 1.0
        biasA = np.zeros((128, 4), f)
        biasA[:, 0] = bq[c01] * scale
        biasA[0:DH, 1] = bq[c2] * scale
        biasA[DH:128, 1] = bq[c2] * scale
        biasA[:, 2] = bk[c01]
        biasA[0:DH, 3] = bk[c2]
        biasA[DH:128, 3] = bk[c2]
        in_maps.append({
            "qT": xT[("q", b)],
            "kT": xT[("k", b)],
            "vT": xT[("v", b)],
            "WqkA": WqkA,
            "WvA": WvA,
            "wo01": np.ascontiguousarray(Wo[c01]).astype(h),
            "wo2": np.ascontiguousarray(Wo[c2]).astype(h),
            "biasA": biasA,
            "padc": np.ascontiguousarray(pad_all[b].reshape(8, 128).T),
            "ones64": np.ones((1, DH), h),
            "emT": np.ascontiguousarray(emT_all[b, h0:h0 + HPC]),
        })
    return in_maps


def get_nc(repeats=1, stage=3):
    key = ("nc", repeats)
    if key not in _CACHE:
        _CACHE[key] = _build(repeats)
    return _CACHE[key]


def assemble(results, bo):
    out = np.zeros((B, SQ, D), np.float32)
    for c in range(N_CORES):
        out[c // GPB] += results[c]["out"].astype(np.float32)
    return out + np.asarray(bo, np.float32)


def kernel(value, key, query, key_padding_mask, attn_mask,
           Wq, Wk, Wv, Wo, bq, bk, bv, bo, **extra):
    from concourse.bass_utils import run_bass_kernel_spmd

    nc = get_nc()
    in_maps = prep_inputs(value, key, query, key_padding_mask, attn_mask,
                          Wq, Wk, Wv, Wo, bq, bk, bv, bo)
    res = run_bass_kernel_spmd(nc, in_maps, core_ids=list(range(N_CORES)),
                               **_CACHE.get("run_kwargs", {}))
    _CACHE["last_results"] = res
    return assemble(res.results, bo)


# revision 3
# speedup vs baseline: 1.5924x; 1.5924x over previous
"""Multi-head attention (B=2, S=1024, D=768, H=12) on 8 TRN2 NeuronCores. v2.

Sharding: batch x head-group. Core c handles batch b = c // 4 and heads
3*(c%4) .. 3*(c%4)+2. Host sums the 4 partials per batch and adds bo.

v2 changes vs v1:
- f16 everywhere on device (qkv, weights, mask, intermediates, out):
  halves HBM traffic and enables 2x DVE mode for the mask op.
- The additive attention mask is shipped as exp(mask) (host precompute)
  and applied as a f16 multiply AFTER the exp (2x DVE) instead of an
  fp32 add on the PSUM logits before it (1x DVE).
- No ones-row in the q/k contraction: biases are added on the PSUM->SBUF
  copies via activation bias (per-partition). V keeps a 1-row ones tile
  (bias + softmax-denominator column), which costs a ~60-cycle matmul.
- Head 2's q/k projections are written twice (duplicated weight columns,
  free on the PE), so every S matmul (contraction 64) can be issued in
  pairs on distinct PE row-groups (0-63 / 64-127) and overlap 2x.
"""

import numpy as np

B, SQ, SK, D, H = 2, 1024, 1024, 768, 12
DH = D // H            # 64
HPC = 3                # heads per core
N_CORES = 8
GPB = 4                # head-groups (cores) per batch
KT = 6                 # k-tiles over contraction dim 768
NEG = -1.0e30

_CACHE = {}


def _build(repeats=1):
    import concourse.tile as tile
    import concourse.mybir as mybir
    from concourse import bacc

    f32 = mybir.dt.float32
    f16 = mybir.dt.float16
    AF = mybir.ActivationFunctionType

    nc = bacc.Bacc("TRN2", target_bir_lowering=False, debug=False,
                   num_devices=N_CORES)

    qT = nc.dram_tensor("qT", [D, SQ], f16, kind="ExternalInput").ap()
    kT = nc.dram_tensor("kT", [D, SK], f16, kind="ExternalInput").ap()
    vT = nc.dram_tensor("vT", [D + 1, SK], f16, kind="ExternalInput").ap()
    # WqkA cols: 0:128 Wq(h01), 128:256 Wq(h2,h2), 256:384 Wk(h01),
    # 384:512 Wk(h2,h2); q cols pre-scaled by 1/sqrt(DH)
    WqkA = nc.dram_tensor("WqkA", [D, 512], f16, kind="ExternalInput").ap()
    # WvA: per j: cols j*65:j*65+64 = Wv(head j), col j*65+64 = ones col;
    # row 768 = bias row (bv | 1)
    WvA = nc.dram_tensor("WvA", [D + 1, 196], f16, kind="ExternalInput").ap()
    wo01_d = nc.dram_tensor("wo01", [128, D], f16, kind="ExternalInput").ap()
    wo2_d = nc.dram_tensor("wo2", [DH, D], f16, kind="ExternalInput").ap()
    # biasA cols: 0 = bq(h01)*scale, 1 = bq(h2,h2)*scale, 2 = bk(h01),
    # 3 = bk(h2,h2)
    biasA = nc.dram_tensor("biasA", [128, 4], f32, kind="ExternalInput").ap()
    padc = nc.dram_tensor("padc", [128, 8], f32, kind="ExternalInput").ap()
    ones64 = nc.dram_tensor("ones64", [1, DH], f16, kind="ExternalInput").ap()
    # exp(mask)^T per head: [HPC, SK, SQ]
    emT = nc.dram_tensor("emT", [HPC, SK, SQ], f16, kind="ExternalInput").ap()
    out_d = nc.dram_tensor("out", [SQ, D], f16, kind="ExternalOutput").ap()

    with tile.TileContext(nc) as tc:
        with (
            tc.tile_pool(name="consts", bufs=1) as cp,
            tc.tile_pool(name="xt", bufs=13) as xtp,
            tc.tile_pool(name="qk", bufs=2) as qkp,
            tc.tile_pool(name="vv", bufs=2) as vvp,
            tc.tile_pool(name="mask", bufs=6) as mkp,
            tc.tile_pool(name="pr", bufs=4) as prp,
            tc.tile_pool(name="pt", bufs=10) as ptp,
            tc.tile_pool(name="norm", bufs=2) as nmp,
            tc.tile_pool(name="tmp", bufs=2) as tmp,
            tc.tile_pool(name="outs", bufs=3) as otp,
            tc.tile_pool(name="ps", bufs=2, space="PSUM") as ps,
            tc.tile_pool(name="cx", bufs=2, space="PSUM") as cxp,
        ):
            # ---- constants (loaded once) ----
            wqk = []
            for t in range(KT):
                w1 = cp.tile([128, 512], f16, tag=f"wqk{t}")
                nc.sync.dma_start(w1[:], WqkA[t * 128:(t + 1) * 128, :])
                wqk.append(w1)
            wv = []
            for t in range(KT):
                w3 = cp.tile([128, 196], f16, tag=f"wv{t}")
                nc.sync.dma_start(w3[:], WvA[t * 128:(t + 1) * 128, :])
                wv.append(w3)
            wvb = cp.tile([1, 196], f16, tag="wvb")
            nc.sync.dma_start(wvb[:], WvA[D:D + 1, :])
            biasT = cp.tile([128, 4], f32, tag="biasT")
            nc.sync.dma_start(biasT[:], biasA)
            o64 = cp.tile([1, DH], f16, tag="o64")
            nc.sync.dma_start(o64[:], ones64)
            wo01 = cp.tile([128, D], f16, tag="wo01")
            nc.sync.dma_start(wo01[:], wo01_d)
            wo2 = cp.tile([DH, D], f16, tag="wo2")
            nc.sync.dma_start(wo2[:], wo2_d)

            def load_x(x_dram, with_ones):
                ts_ = []
                for t in range(KT):
                    xt_t = xtp.tile([128, SQ], f16, tag="xt")
                    nc.sync.dma_start(xt_t[:], x_dram[t * 128:(t + 1) * 128, :])
                    ts_.append(xt_t)
                if with_ones:
                    xo = xtp.tile([1, SQ], f16, tag="xones")
                    nc.sync.dma_start(xo[:], x_dram[D:D + 1, :])
                    ts_.append(xo)
                return ts_

            for _rep in range(repeats):
                pad = tmp.tile([128, 8], f32, tag="pad")
                nc.sync.dma_start(pad[:], padc)

                # ---- q/k projections -> [128, SQ] f16, channel-partition ----
                def proj_one(xts, col0, bcol, tag):
                    dst = qkp.tile([128, SQ], f16, tag=tag)
                    pps = ps.tile([128, SQ], f32, tag="ps")
                    for t in range(KT):
                        lhs = wqk[t][:, col0: col0 + 128]
                        for n in range(2):
                            nc.tensor.matmul(
                                pps[:, n * 512:(n + 1) * 512],
                                lhs, xts[t][:, n * 512:(n + 1) * 512],
                                start=(t == 0), stop=(t == KT - 1))
                    nc.scalar.add(dst[:], pps[:], biasT[:, bcol:bcol + 1])
                    return dst

                qx = load_x(qT, False)
                q_c0 = proj_one(qx, 0, 0, "q0")
                q_c1 = proj_one(qx, 128, 1, "q1")
                kx = load_x(kT, False)
                k_c0 = proj_one(kx, 256, 2, "k0")
                k_c1 = proj_one(kx, 384, 3, "k1")

                # ---- v projection -> 8 x [128 tok, 196 ch] f16 ----
                vx = load_x(vT, True)
                vtiles = []
                for i in range(8):
                    vps = ps.tile([128, 196], f32, tag="ps")
                    for t in range(KT):
                        nc.tensor.matmul(
                            vps[:], vx[t][:, i * 128:(i + 1) * 128], wv[t][:],
                            start=(t == 0), stop=False)
                    nc.tensor.matmul(
                        vps[:], vx[KT][:, i * 128:(i + 1) * 128], wvb[:],
                        start=False, stop=True)
                    vt = vvp.tile([128, 196], f16, tag=f"v{i}")
                    nc.vector.tensor_copy(vt[:], vps[:])
                    vtiles.append(vt)

                # ---- attention ----
                # rows(j): which 64-row group head j's q/k slice lives on,
                # per i-tile. h0: rows 0:64 of q_c0; h1: rows 64:128 of
                # q_c0; h2: alternates between halves of q_c1 (duplicated).
                def qk_slice(j, i):
                    if j == 0:
                        return q_c0[0:DH, :], k_c0[0:DH, :]
                    if j == 1:
                        return q_c0[DH:128, :], k_c0[DH:128, :]
                    if i % 2 == 0:
                        return q_c1[0:DH, :], k_c1[0:DH, :]
                    return q_c1[DH:128, :], k_c1[DH:128, :]

                pts = {}

                def emit_tile(j, i):
                    qs, ks = qk_slice(j, i)
                    sps = ps.tile([128, SQ], f32, tag="ps")
                    for n in range(2):
                        nc.tensor.matmul(
                            sps[:, n * 512:(n + 1) * 512],
                            ks[:, i * 128:(i + 1) * 128],
                            qs[:, n * 512:(n + 1) * 512],
                            start=True, stop=True)
                    ptr = prp.tile([128, SQ], f16, tag="ptr")
                    nc.scalar.activation(ptr[:], sps[:], AF.Exp,
                                         bias=pad[:, i:i + 1], scale=1.0)
                    mk = mkp.tile([128, SQ], f16, tag="mask")
                    nc.sync.dma_start(mk[:], emT[j, i * 128:(i + 1) * 128, :])
                    pt = ptp.tile([128, SQ], f16, tag="pt")
                    nc.vector.tensor_mul(pt[:], ptr[:], mk[:])
                    pts[(j, i)] = pt

                def emit_av(ctx_, j_, i_):
                    pt_ = pts[(j_, i_)]
                    for n in range(2):
                        nc.tensor.matmul(
                            ctx_[:, n * 512:(n + 1) * 512],
                            vtiles[i_][:, j_ * 65:j_ * 65 + 65],
                            pt_[:, n * 512:(n + 1) * 512],
                            start=(i_ == 0), stop=(i_ == 7))
                    del pts[(j_, i_)]

                cn01 = nmp.tile([128, SQ], f16, tag="cn01")
                cn2 = nmp.tile([DH, SQ], f16, tag="cn2")

                def emit_norm(j, ctx):
                    srow = tmp.tile([1, SQ], f16, tag="srow")
                    nc.scalar.copy(srow[:], ctx[DH:DH + 1, :])
                    rb = ps.tile([DH, SQ], f32, tag="ps")
                    for n in range(2):
                        nc.tensor.matmul(rb[:, n * 512:(n + 1) * 512],
                                         o64[:], srow[:, n * 512:(n + 1) * 512],
                                         start=True, stop=True)
                    rbr = tmp.tile([DH, SQ], f32, tag="rbr")
                    nc.vector.reciprocal(rbr[:], rb[:])
                    if j < 2:
                        nc.vector.tensor_mul(cn01[j * DH:(j + 1) * DH, :],
                                             ctx[0:DH, :], rbr[:])
                    else:
                        nc.vector.tensor_mul(cn2[:], ctx[0:DH, :], rbr[:])

                ctx0 = cxp.tile([65, SQ], f32, tag="cx")
                ctx1 = cxp.tile([65, SQ], f32, tag="cx")
                # h0/h1 interleaved: S pairs land on distinct PE row groups
                for i in range(8):
                    emit_tile(0, i)
                    emit_tile(1, i)
                    if i > 0:
                        emit_av(ctx0, 0, i - 1)
                        emit_av(ctx1, 1, i - 1)
                emit_av(ctx0, 0, 7)
                emit_av(ctx1, 1, 7)
                emit_norm(0, ctx0)
                emit_norm(1, ctx1)
                # h2: alternating row groups via duplicated q_c1/k_c1
                ctx2 = cxp.tile([65, SQ], f32, tag="cx")
                for i in range(8):
                    emit_tile(2, i)
                    if i > 0:
                        emit_av(ctx2, 2, i - 1)
                emit_av(ctx2, 2, 7)
                emit_norm(2, ctx2)

                # ---- output projection ----
                for t in range(8):
                    # outproj psum in the cx ring: frees the "ps" ring's
                    # tail so the next rep's projection psums allocate
                    # before this rep's outproj completes (phase overlap)
                    ops = cxp.tile([128, D], f32, tag="cx")
                    for n0, nw in ((0, 512), (512, 256)):
                        nc.tensor.matmul(
                            ops[:, n0:n0 + nw],
                            cn01[:, t * 128:(t + 1) * 128],
                            wo01[:, n0:n0 + nw],
                            start=True, stop=False)
                        nc.tensor.matmul(
                            ops[:, n0:n0 + nw],
                            cn2[:, t * 128:(t + 1) * 128],
                            wo2[:, n0:n0 + nw],
                            start=False, stop=True)
                    ot = otp.tile([128, D], f16, tag=f"ot{t % 3}")
                    nc.vector.tensor_copy(ot[:], ops[:])
                    nc.sync.dma_start(out_d[t * 128:(t + 1) * 128, :], ot[:])

    nc.compile()
    return nc


def prep_inputs(value, key, query, key_padding_mask, attn_mask,
                Wq, Wk, Wv, Wo, bq, bk, bv, bo):
    f = np.float32
    h = np.float16
    value = np.asarray(value, f)
    key = np.asarray(key, f)
    query = np.asarray(query, f)
    key_padding_mask = np.asarray(key_padding_mask)
    attn_mask = np.asarray(attn_mask, f)
    Wq, Wk, Wv, Wo = (np.asarray(w, f) for w in (Wq, Wk, Wv, Wo))
    bq, bk, bv = (np.asarray(x, f) for x in (bq, bk, bv))

    scale = f(1.0 / np.sqrt(DH))
    ones_row = np.ones((1, SK), h)
    xT = {}
    for b in range(B):
        xT[("q", b)] = np.ascontiguousarray(query[b].T).astype(h)
        xT[("k", b)] = np.ascontiguousarray(key[b].T).astype(h)
        xT[("v", b)] = np.concatenate(
            [np.ascontiguousarray(value[b].T).astype(h), ones_row])
    emT_all = np.exp(attn_mask.transpose(0, 1, 3, 2)).astype(h)
    pad_all = np.where(key_padding_mask, f(0), f(NEG)).astype(f)  # [B, SK]

    in_maps = []
    for c in range(N_CORES):
        b, g = divmod(c, GPB)
        h0 = g * HPC
        c01 = slice(h0 * DH, (h0 + 2) * DH)
        c2 = slice((h0 + 2) * DH, (h0 + 3) * DH)
        WqkA = np.zeros((D, 512), h)
        WqkA[:, 0:128] = (Wq[:, c01] * scale).astype(h)
        WqkA[:, 128:192] = (Wq[:, c2] * scale).astype(h)
        WqkA[:, 192:256] = WqkA[:, 128:192]
        WqkA[:, 256:384] = Wk[:, c01].astype(h)
        WqkA[:, 384:448] = Wk[:, c2].astype(h)
        WqkA[:, 448:512] = WqkA[:, 384:448]
        WvA = np.zeros((D + 1, 196), h)
        for j in range(HPC):
            hc = slice((h0 + j) * DH, (h0 + j + 1) * DH)
            WvA[:D, j * 65:j * 65 + DH] = Wv[:, hc].astype(h)
            WvA[D, j * 65:j * 65 + DH] = bv[hc].astype(h)
            WvA[D, j * 65 + DH] = 1.0
        biasA = np.zeros((128, 4), f)
        biasA[:, 0] = bq[c01] * scale
        biasA[0:DH, 1] = bq[c2] * scale
        biasA[DH:128, 1] = bq[c2] * scale
        biasA[:, 2] = bk[c01]
        biasA[0:DH, 3] = bk[c2]
        biasA[DH:128, 3] = bk[c2]
        in_maps.append({
            "qT": xT[("q", b)],
            "kT": xT[("k", b)],
            "vT": xT[("v", b)],
            "WqkA": WqkA,
            "WvA": WvA,
            "wo01": np.ascontiguousarray(Wo[c01]).astype(h),
            "wo2": np.ascontiguousarray(Wo[c2]).astype(h),
            "biasA": biasA,
            "padc": np.ascontiguousarray(pad_all[b].reshape(8, 128).T),
            "ones64": np.ones((1, DH), h),
            "emT": np.ascontiguousarray(emT_all[b, h0:h0 + HPC]),
        })
    return in_maps


def get_nc(repeats=1, stage=3):
    key = ("nc", repeats)
    if key not in _CACHE:
        _CACHE[key] = _build(repeats)
    return _CACHE[key]


def assemble(results, bo):
    out = np.zeros((B, SQ, D), np.float32)
    for c in range(N_CORES):
        out[c // GPB] += results[c]["out"].astype(np.float32)
    return out + np.asarray(bo, np.float32)


def kernel(value, key, query, key_padding_mask, attn_mask,
           Wq, Wk, Wv, Wo, bq, bk, bv, bo, **extra):
    from concourse.bass_utils import run_bass_kernel_spmd

    nc = get_nc()
    in_maps = prep_inputs(value, key, query, key_padding_mask, attn_mask,
                          Wq, Wk, Wv, Wo, bq, bk, bv, bo)
    res = run_bass_kernel_spmd(nc, in_maps, core_ids=list(range(N_CORES)),
                               **_CACHE.get("run_kwargs", {}))
    _CACHE["last_results"] = res
    return assemble(res.results, bo)
